# revision 6
# baseline (speedup 1.0000x reference)
"""AttentionPooling kernel for Trainium2 (8 NeuronCores, SPMD, no collectives).

reference math:
    scores = tanh(x @ W1 + b1) @ W2 + b2        # [N, 1]
    attn   = softmax(scores, axis=0)            # global over all N rows
    pooled = segment_sum(x * attn, batch, 1024) # [1024, 256]

Strategy (v2, bf16):
  - batch is sorted, so shard ROWS at graph boundaries: core c gets all rows
    with batch in [128c, 128(c+1)).  Each core owns exactly 128 output graphs
    -> no cross-core reduction for pooled.
  - b2 cancels in softmax (constant shift) -> dropped.
  - softmax normalizer: each core returns unnormalized A_g = sum_i e_i x_i and
    per-row e values; host divides by the global Z (exact).
  - x is DMA'd HBM->SBUF with an on-the-fly fp32->bf16 cast (SWDGE), so all
    PE work runs in bf16 (FWL weight loads, 1 cyc/row matmuls).
  - h is computed with rows i on PSUM partitions (lhsT = xT tile chunks), so
    the score dot product s_i = w2 . tanh(h_i) is a DVE multiply-reduce along
    the free dim -- no PE score matmuls, and e lands as a [128,1] column.
  - per 128-row tile:
      xT   = transpose(x_tile) on PE (bf16)     (PSUM->SBUF copy DVE+ACT)
      h    = sum_dc xT_dc^T @ W1[dc]            (PE, rhs = W1 moving)
      th   = tanh(h) on ACT -> bf16 SBUF
      s    = reduce_add(th * w2rep) on DVE -> [128,1]
      e    = exp(s) on ACT (batched per 2 tiles) -> evec column
      m    = (iota == brel) * e   one fused DVE tensor_scalar -> bf16
      acc[128g, 256] += m^T @ x_tile   (PE, PSUM-resident accumulator)
"""

import numpy as np
from contextlib import ExitStack

import concourse.bass as bass
import concourse.bacc as bacc
import concourse.mybir as mybir
import concourse.tile as tile
from concourse.bass_utils import run_bass_kernel_spmd
from concourse.masks import make_identity

F32 = mybir.dt.float32
BF16 = mybir.dt.bfloat16
I32 = mybir.dt.int32

NUM_GRAPHS = 1024
NC = 8
GPC = NUM_GRAPHS // NC  # graphs per core = 128
P = 128
D = 256
ST = 8  # tiles per DMA supertile (1 MiB fp32 read per chunk)


def build_program(R: int, T: int, with_b1: bool) -> bass.Bass:
    assert T % ST == 0 and R == T * P
    nsup = T // ST

    nc = bacc.Bacc("TRN2", target_bir_lowering=False, debug=False)
    xs = nc.declare_dram_parameter("xs", [R, D], F32, isOutput=False)
    brel = nc.declare_dram_parameter("brel", [P, T], F32, isOutput=False)
    w1 = nc.declare_dram_parameter("w1", [D, D], F32, isOutput=False)
    w2r = nc.declare_dram_parameter("w2r", [P, D], F32, isOutput=False)
    if with_b1:
        b1d = nc.declare_dram_parameter("b1d", [1, D], F32, isOutput=False)
    pooled = nc.declare_dram_parameter("pooled", [P, D], F32, isOutput=True)
    evec_out = nc.declare_dram_parameter("evec_out", [P, T], F32, isOutput=True)

    with ExitStack() as ctx:
        tc = ctx.enter_context(tile.TileContext(nc))
        const = ctx.enter_context(tc.tile_pool(name="const", bufs=1))
        xpool = ctx.enter_context(tc.tile_pool(name="x", bufs=3))
        xtpp = ctx.enter_context(tc.tile_pool(name="xtp", bufs=3, space="PSUM"))
        xtsp = ctx.enter_context(tc.tile_pool(name="xts", bufs=3))
        hpp = ctx.enter_context(tc.tile_pool(name="hp", bufs=2, space="PSUM"))
        thp = ctx.enter_context(tc.tile_pool(name="th", bufs=2))
        scrp = ctx.enter_context(tc.tile_pool(name="scr", bufs=2))
        spp = ctx.enter_context(tc.tile_pool(name="sp", bufs=2))
        mpl = ctx.enter_context(tc.tile_pool(name="m", bufs=3))
        accp = ctx.enter_context(tc.tile_pool(name="acc", bufs=1, space="PSUM"))
        outp = ctx.enter_context(tc.tile_pool(name="out", bufs=1))

        # ---- constants ----
        identf = const.tile([P, P], F32, tag="identf")
        make_identity(nc, identf[:])
        ident = const.tile([P, P], BF16)
        nc.vector.tensor_copy(ident[:], identf[:])
        iota_i = const.tile([P, P], I32)
        nc.gpsimd.iota(iota_i[:], pattern=[[1, P]], base=0, channel_multiplier=0)
        iota_f = const.tile([P, P], F32)
        nc.vector.tensor_copy(iota_f[:], iota_i[:])

        w1f = const.tile([P, 2, D], F32, tag="w1f")  # [d_lo, dc, j]
        nc.sync.dma_start(w1f[:], w1.rearrange("(dc p) j -> p dc j", p=P))
        w1sb = const.tile([P, 2, D], BF16)
        nc.vector.tensor_copy(w1sb[:], w1f[:])
        w2f = const.tile([P, D], F32, tag="w2f")  # w2[j] replicated on partitions
        nc.sync.dma_start(w2f[:], w2r[:])
        w2rep = const.tile([P, D], BF16)
        nc.vector.tensor_copy(w2rep[:], w2f[:])
        brelsb = const.tile([P, T], F32)
        nc.sync.dma_start(brelsb[:], brel[:])
        if with_b1:
            b1f = const.tile([1, D], F32, tag="b1f")  # [1, j]
            nc.sync.dma_start(b1f[:], b1d[:])
            b1sb = const.tile([1, D], BF16)
            nc.vector.tensor_copy(b1sb[:], b1f[:])
            ones_cf = const.tile([1, P], F32, tag="ones_cf")
            nc.gpsimd.memset(ones_cf[:], 1.0)
            ones_col = const.tile([1, P], BF16)
            nc.vector.tensor_copy(ones_col[:], ones_cf[:])

        evec = const.tile([P, T], F32, tag="evec")  # exp(s) per row
        # persistent PSUM accumulator
        acc = accp.tile([P, D], F32)  # pooled[g, d]

        Tanh = mybir.ActivationFunctionType.Tanh
        Exp = mybir.ActivationFunctionType.Exp

        for sup in range(nsup):
            xsb = xpool.tile([P, ST, D], BF16)
            src = xs[sup * ST * P : (sup + 1) * ST * P, :]
            nc.gpsimd.dma_start(xsb[:], src.rearrange("(p t) d -> p t d", p=P))

            for pair in range(ST // 2):
                spair = spp.tile([P, 2], F32)
                ths = []
                for tt in range(2):
                    t = pair * 2 + tt
                    gt = sup * ST + t
                    # transpose x_tile -> xT chunks in PSUM (bf16)
                    xtp = xtpp.tile([P, 2, P], BF16)
                    for dc in range(2):
                        nc.tensor.transpose(
                            xtp[:, dc, :],
                            xsb[:, t, dc * P : (dc + 1) * P],
                            ident[:],
                        )
                    xts = xtsp.tile([P, 2, P], BF16)
                    nc.vector.tensor_copy(xts[:, 0], xtp[:, 0])
                    nc.scalar.copy(xts[:, 1], xtp[:, 1])

                    # h[i, j] = sum_d x[i, d] W1[d, j]  (i on partitions)
                    hp = hpp.tile([P, D], F32)
                    for dc in range(2):
                        nc.tensor.matmul(
                            hp[:],
                            lhsT=xts[:, dc],
                            rhs=w1sb[:, dc],
                            start=(dc == 0),
                            stop=(dc == 1 and not with_b1),
                        )
                    if with_b1:
                        nc.tensor.matmul(
                            hp[:],
                            lhsT=ones_col[:],
                            rhs=b1sb[:],
                            start=False,
                            stop=True,
                        )
                    th = thp.tile([P, D], BF16)
                    nc.scalar.activation(th[:], hp[:], Tanh)
                    ths.append(th)

                    # s_i = sum_j th[i, j] * w2[j]  (DVE multiply then reduce)
                    scr = scrp.tile([P, D], BF16)
                    nc.vector.tensor_tensor(
                        scr[:], th[:], w2rep[:], op=mybir.AluOpType.mult
                    )
                    nc.vector.tensor_reduce(
                        spair[:, tt : tt + 1],
                        scr[:],
                        axis=mybir.AxisListType.X,
                        op=mybir.AluOpType.add,
                    )

                gt0 = sup * ST + pair * 2
                nc.scalar.activation(evec[:, gt0 : gt0 + 2], spair[:], Exp)

                for tt in range(2):
                    t = pair * 2 + tt
                    gt = gt0 + tt
                    m = mpl.tile([P, P], BF16)
                    nc.vector.tensor_scalar(
                        m[:],
                        iota_f[:],
                        brelsb[:, gt : gt + 1],
                        evec[:, gt : gt + 1],
                        op0=mybir.AluOpType.is_equal,
                        op1=mybir.AluOpType.mult,
                    )
                    nc.tensor.matmul(
                        acc[:],
                        lhsT=m[:],
                        rhs=xsb[:, t, :],
                        start=(gt == 0),
                        stop=(gt == T - 1),
                        skip_group_check=True,
                    )

        out_sb = outp.tile([P, D], F32)
        nc.vector.tensor_copy(out_sb[:], acc[:])
        nc.sync.dma_start(pooled[:], out_sb[:])
        nc.sync.dma_start(evec_out[:], evec[:])

    nc.compile()
    return nc


def _prep_inputs(x, batch, W1, b1, W2):
    """Shard rows at graph boundaries; pad to a common multiple of ST*P rows."""
    x = np.ascontiguousarray(np.asarray(x, dtype=np.float32))
    batch = np.asarray(batch)
    bounds = np.searchsorted(batch, np.arange(0, NUM_GRAPHS + 1, GPC))
    counts = np.diff(bounds)
    chunk = ST * P
    R = int(np.ceil(max(int(counts.max()), 1) / chunk) * chunk)
    T = R // P

    w1h = np.ascontiguousarray(np.asarray(W1, dtype=np.float32))  # [d, j]
    w2h = np.ascontiguousarray(
        np.broadcast_to(np.asarray(W2, dtype=np.float32).reshape(1, D), (P, D))
    )  # w2 replicated across partitions
    b1h = np.asarray(b1, dtype=np.float32).reshape(1, D)
    with_b1 = bool(np.any(b1h))

    in_maps = []
    for c in range(NC):
        lo, hi = int(bounds[c]), int(bounds[c + 1])
        n = hi - lo
        xs = np.zeros((R, D), dtype=np.float32)
        xs[:n] = x[lo:hi]
        br = np.full((R,), -1.0, dtype=np.float32)
        br[:n] = (np.asarray(batch[lo:hi], dtype=np.int64) - c * GPC).astype(
            np.float32
        )
        # row r = sup*(P*ST) + p*ST + t lives at brel_pt[p, sup*ST + t]
        nsup = T // ST
        brel_pt = np.ascontiguousarray(
            br.reshape(nsup, P, ST).transpose(1, 0, 2).reshape(P, T)
        )
        m = {"xs": xs, "brel": brel_pt, "w1": w1h, "w2r": w2h}
        if with_b1:
            m["b1d"] = b1h
        in_maps.append(m)
    return in_maps, R, T, with_b1, [int(c) for c in counts]


def run(x, batch, W1, b1, W2, b2, trace=False, trace_kwargs=None):
    in_maps, R, T, with_b1, counts = _prep_inputs(x, batch, W1, b1, W2)
    nc = build_program(R, T, with_b1)
    res = run_bass_kernel_spmd(
        nc,
        in_maps,
        core_ids=list(range(NC)),
        trace=trace,
        **(trace_kwargs or {}),
    )
    A = np.concatenate(
        [res.results[c]["pooled"] for c in range(NC)], axis=0
    ).astype(np.float64)
    nsup = T // ST
    Z = 0.0
    for c in range(NC):
        ev = res.results[c]["evec_out"].astype(np.float64)  # [P, T]
        n = counts[c]
        # row r = sup*(P*ST) + p*ST + t is at ev[p, sup*ST + t]
        rows = ev.reshape(P, nsup, ST).transpose(1, 0, 2).reshape(-1)
        Z += rows[:n].sum()
    out = (A / Z).astype(np.float32)
    return out, res


def kernel(x, batch, W1, b1, W2, b2):
    out, _ = run(x, batch, W1, b1, W2, b2)
    return out


# revision 28
# speedup vs baseline: 1.2996x; 1.2996x over previous
"""AttentionPooling kernel for Trainium2 (8 NeuronCores, SPMD, no collectives).

reference math:
    scores = tanh(x @ W1 + b1) @ W2 + b2        # [N, 1]
    attn   = softmax(scores, axis=0)            # global over all N rows
    pooled = segment_sum(x * attn, batch, 1024) # [1024, 256]

Strategy (v2, bf16):
  - batch is sorted, so shard ROWS at graph boundaries: core c gets all rows
    with batch in [128c, 128(c+1)).  Each core owns exactly 128 output graphs
    -> no cross-core reduction for pooled.
  - b2 cancels in softmax (constant shift) -> dropped.
  - softmax normalizer: each core returns unnormalized A_g = sum_i e_i x_i and
    per-row e values; host divides by the global Z (exact).
  - x is DMA'd HBM->SBUF with an on-the-fly fp32->bf16 cast (SWDGE), so all
    PE work runs in bf16 (FWL weight loads, 1 cyc/row matmuls).
  - h is computed with rows i on PSUM partitions (lhsT = xT tile chunks), so
    the score dot product s_i = w2 . tanh(h_i) is a DVE multiply-reduce along
    the free dim -- no PE score matmuls, and e lands as a [128,1] column.
  - per 128-row tile:
      xT   = transpose(x_tile) on PE (bf16)     (PSUM->SBUF copy DVE+ACT)
      h    = sum_dc xT_dc^T @ W1[dc]            (PE, rhs = W1 moving)
      th   = tanh(h) on ACT -> bf16 SBUF
      s    = reduce_add(th * w2rep) on DVE -> [128,1]
      e    = exp(s) on ACT (batched per 2 tiles) -> evec column
      m    = (iota == brel) * e   one fused DVE tensor_scalar -> bf16
      acc[128g, 256] += m^T @ x_tile   (PE, PSUM-resident accumulator)
"""

import numpy as np
from contextlib import ExitStack

import concourse.bass as bass
import concourse.bacc as bacc
import concourse.mybir as mybir
import concourse.tile as tile
from concourse.bass_utils import run_bass_kernel_spmd
from concourse.masks import make_identity

F32 = mybir.dt.float32
BF16 = mybir.dt.bfloat16
I32 = mybir.dt.int32

NUM_GRAPHS = 1024
NC = 8
GPC = NUM_GRAPHS // NC  # graphs per core = 128
P = 128
D = 256
ST = 8  # tiles per DMA supertile (1 MiB fp32 read per chunk)
G = 4  # tiles per compute group (tanh/exp batching)


def build_program(R: int, T: int, with_b1: bool) -> bass.Bass:
    assert T % ST == 0 and R == T * P
    nsup = T // ST

    nc = bacc.Bacc("TRN2", target_bir_lowering=False, debug=False)
    xs = nc.declare_dram_parameter("xs", [R, D], F32, isOutput=False)
    brel = nc.declare_dram_parameter("brel", [P, T], F32, isOutput=False)
    w1 = nc.declare_dram_parameter("w1", [D, D], F32, isOutput=False)
    w2r = nc.declare_dram_parameter("w2r", [P, D], F32, isOutput=False)
    if with_b1:
        b1d = nc.declare_dram_parameter("b1d", [1, D], F32, isOutput=False)
    pooled = nc.declare_dram_parameter("pooled", [P, D], F32, isOutput=True)
    evec_out = nc.declare_dram_parameter("evec_out", [P, T], F32, isOutput=True)

    with ExitStack() as ctx:
        tc = ctx.enter_context(tile.TileContext(nc))
        const = ctx.enter_context(tc.tile_pool(name="const", bufs=1))
        xpool = ctx.enter_context(tc.tile_pool(name="x", bufs=3))
        xtpp = ctx.enter_context(tc.tile_pool(name="xtp", bufs=3, space="PSUM"))
        xtsp = ctx.enter_context(tc.tile_pool(name="xts", bufs=5))
        hpp = ctx.enter_context(tc.tile_pool(name="hp", bufs=2, space="PSUM"))
        thp = ctx.enter_context(tc.tile_pool(name="th", bufs=3))
        scrp = ctx.enter_context(tc.tile_pool(name="scr", bufs=4))
        spp = ctx.enter_context(tc.tile_pool(name="sp", bufs=4))
        mpl = ctx.enter_context(tc.tile_pool(name="m", bufs=8))
        uqp = ctx.enter_context(tc.tile_pool(name="uq", bufs=2))
        accp = ctx.enter_context(tc.tile_pool(name="acc", bufs=1, space="PSUM"))
        outp = ctx.enter_context(tc.tile_pool(name="out", bufs=1))

        # ---- constants ----
        identf = const.tile([P, P], F32, tag="identf")
        make_identity(nc, identf[:])
        ident = const.tile([P, P], BF16)
        nc.vector.tensor_copy(ident[:], identf[:])
        iota_i = const.tile([P, P], I32)
        nc.gpsimd.iota(iota_i[:], pattern=[[1, P]], base=0, channel_multiplier=0)
        iota_b = const.tile([P, P], BF16)
        nc.vector.tensor_copy(iota_b[:], iota_i[:])

        w1f = const.tile([P, 2, D], F32, tag="w1f")  # [d_lo, dc, j]
        nc.sync.dma_start(w1f[:], w1.rearrange("(dc p) j -> p dc j", p=P))
        w1sb = const.tile([P, 2, D], BF16)
        nc.vector.tensor_copy(w1sb[:], w1f[:])
        w2f = const.tile([P, D], F32, tag="w2f")  # w2[j] replicated on partitions
        nc.sync.dma_start(w2f[:], w2r[:])
        w2rep = const.tile([P, D], BF16)
        nc.vector.tensor_copy(w2rep[:], w2f[:])
        brelsb = const.tile([P, T], F32)
        nc.sync.dma_start(brelsb[:], brel[:])
        if with_b1:
            b1f = const.tile([1, D], F32, tag="b1f")  # [1, j]
            nc.sync.dma_start(b1f[:], b1d[:])
            b1sb = const.tile([1, D], BF16)
            nc.vector.tensor_copy(b1sb[:], b1f[:])
            ones_cf = const.tile([1, P], F32, tag="ones_cf")
            nc.gpsimd.memset(ones_cf[:], 1.0)
            ones_col = const.tile([1, P], BF16)
            nc.vector.tensor_copy(ones_col[:], ones_cf[:])

        evec = const.tile([P, T], F32, tag="evec")  # exp(s) per row
        # negated brel for the ACT-side mask build (bias = -brel)
        negb = const.tile([P, T], F32, tag="negb")
        nc.vector.tensor_scalar(
            negb[:], brelsb[:], -1.0, None, op0=mybir.AluOpType.mult
        )
        # persistent PSUM accumulator
        acc = accp.tile([P, D], F32)  # pooled[g, d]

        Tanh = mybir.ActivationFunctionType.Tanh
        Exp = mybir.ActivationFunctionType.Exp
        Square = mybir.ActivationFunctionType.Square

        def emit_macc(pend):
            """m-build + acc matmuls for a completed group. Every 4th tile
            builds the mask on ACT instead of DVE (load balancing):
            m = Exp(-60*(iota-brel)^2 + s) == onehot * e exactly enough."""
            gt0_, xsb_, grp_, s4_ = pend
            for tt in range(G):
                t = grp_ * G + tt
                gt = gt0_ + tt
                m = mpl.tile([P, P], BF16)
                if tt == 3:
                    uq = uqp.tile([P, P], BF16)
                    nc.scalar.activation(
                        uq[:], iota_b[:], Square, bias=negb[:, gt : gt + 1]
                    )
                    nc.scalar.activation(
                        m[:], uq[:], Exp, bias=s4_[:, tt : tt + 1], scale=-60.0
                    )
                else:
                    nc.vector.tensor_scalar(
                        m[:],
                        iota_b[:],
                        brelsb[:, gt : gt + 1],
                        evec[:, gt : gt + 1],
                        op0=mybir.AluOpType.is_equal,
                        op1=mybir.AluOpType.mult,
                    )
                nc.tensor.matmul(
                    acc[:],
                    lhsT=m[:],
                    rhs=xsb_[:, t, :],
                    start=(gt == 0),
                    stop=(gt == T - 1),
                    skip_group_check=True,
                )

        for sup in range(nsup):
            xsb = xpool.tile([P, ST, D], BF16)
            src = xs[sup * ST * P : (sup + 1) * ST * P, :]
            nc.gpsimd.dma_start(xsb[:], src.rearrange("(p t) d -> p t d", p=P))

            for grp in range(ST // G):
                s4 = spp.tile([P, G], F32)
                h4 = hpp.tile([P, G, D], F32)
                th4 = thp.tile([P, G, D], BF16)
                for tt in range(G):
                    t = grp * G + tt
                    # transpose x_tile -> xT chunks in PSUM (bf16)
                    xtp = xtpp.tile([P, 2, P], BF16)
                    for dc in range(2):
                        nc.tensor.transpose(
                            xtp[:, dc, :],
                            xsb[:, t, dc * P : (dc + 1) * P],
                            ident[:],
                        )
                    xts = xtsp.tile([P, 2, P], BF16)
                    nc.vector.tensor_copy(xts[:], xtp[:])

                    # h[i, j] = sum_d x[i, d] W1[d, j]  (i on partitions)
                    for dc in range(2):
                        nc.tensor.matmul(
                            h4[:, tt, :],
                            lhsT=xts[:, dc],
                            rhs=w1sb[:, dc],
                            start=(dc == 0),
                            stop=(dc == 1 and not with_b1),
                        )
                    if with_b1:
                        nc.tensor.matmul(
                            h4[:, tt, :],
                            lhsT=ones_col[:],
                            rhs=b1sb[:],
                            start=False,
                            stop=True,
                        )
                # tanh per PSUM bank (2 tiles = 2KB; a 2-bank AP reads wrong)
                nc.scalar.activation(th4[:, 0:2], h4[:, 0:2], Tanh)
                nc.scalar.activation(th4[:, 2:4], h4[:, 2:4], Tanh)

                for tt in range(G):
                    # s_i = sum_j th[i, j] * w2[j]: fused mult+reduce on DVE
                    scr = scrp.tile([P, D], BF16)
                    nc.vector.scalar_tensor_tensor(
                        scr[:],
                        th4[:, tt, :],
                        1.0,
                        w2rep[:],
                        op0=mybir.AluOpType.mult,
                        op1=mybir.AluOpType.mult,
                        accum_out=s4[:, tt : tt + 1],
                    )

                gt0 = sup * ST + grp * G
                nc.scalar.activation(evec[:, gt0 : gt0 + G], s4[:], Exp)

                emit_macc((gt0, xsb, grp, s4))

        out_sb = outp.tile([P, D], F32)
        nc.vector.tensor_copy(out_sb[:], acc[:])
        nc.sync.dma_start(pooled[:], out_sb[:])
        nc.sync.dma_start(evec_out[:], evec[:])

    nc.compile()
    return nc


def _prep_inputs(x, batch, W1, b1, W2):
    """Shard rows at graph boundaries; pad to a common multiple of ST*P rows."""
    x = np.ascontiguousarray(np.asarray(x, dtype=np.float32))
    batch = np.asarray(batch)
    bounds = np.searchsorted(batch, np.arange(0, NUM_GRAPHS + 1, GPC))
    counts = np.diff(bounds)
    chunk = ST * P
    R = int(np.ceil(max(int(counts.max()), 1) / chunk) * chunk)
    T = R // P

    w1h = np.ascontiguousarray(np.asarray(W1, dtype=np.float32))  # [d, j]
    w2h = np.ascontiguousarray(
        np.broadcast_to(np.asarray(W2, dtype=np.float32).reshape(1, D), (P, D))
    )  # w2 replicated across partitions
    b1h = np.asarray(b1, dtype=np.float32).reshape(1, D)
    with_b1 = bool(np.any(b1h))

    in_maps = []
    for c in range(NC):
        lo, hi = int(bounds[c]), int(bounds[c + 1])
        n = hi - lo
        xs = np.zeros((R, D), dtype=np.float32)
        xs[:n] = x[lo:hi]
        br = np.full((R,), -1.0, dtype=np.float32)
        br[:n] = (np.asarray(batch[lo:hi], dtype=np.int64) - c * GPC).astype(
            np.float32
        )
        # row r = sup*(P*ST) + p*ST + t lives at brel_pt[p, sup*ST + t]
        nsup = T // ST
        brel_pt = np.ascontiguousarray(
            br.reshape(nsup, P, ST).transpose(1, 0, 2).reshape(P, T)
        )
        m = {"xs": xs, "brel": brel_pt, "w1": w1h, "w2r": w2h}
        if with_b1:
            m["b1d"] = b1h
        in_maps.append(m)
    return in_maps, R, T, with_b1, [int(c) for c in counts]


def run(x, batch, W1, b1, W2, b2, trace=False, trace_kwargs=None):
    in_maps, R, T, with_b1, counts = _prep_inputs(x, batch, W1, b1, W2)
    nc = build_program(R, T, with_b1)
    res = run_bass_kernel_spmd(
        nc,
        in_maps,
        core_ids=list(range(NC)),
        trace=trace,
        **(trace_kwargs or {}),
    )
    A = np.concatenate(
        [res.results[c]["pooled"] for c in range(NC)], axis=0
    ).astype(np.float64)
    nsup = T // ST
    Z = 0.0
    for c in range(NC):
        ev = res.results[c]["evec_out"].astype(np.float64)  # [P, T]
        n = counts[c]
        # row r = sup*(P*ST) + p*ST + t is at ev[p, sup*ST + t]
        rows = ev.reshape(P, nsup, ST).transpose(1, 0, 2).reshape(-1)
        Z += rows[:n].sum()
    out = (A / Z).astype(np.float32)
    return out, res


def kernel(x, batch, W1, b1, W2, b2):
    out, _ = run(x, batch, W1, b1, W2, b2)
    return out


# revision 29
# speedup vs baseline: 1.5370x; 1.1826x over previous
"""AttentionPooling kernel for Trainium2 (8 NeuronCores, SPMD, no collectives).

reference math:
    scores = tanh(x @ W1 + b1) @ W2 + b2        # [N, 1]
    attn   = softmax(scores, axis=0)            # global over all N rows
    pooled = segment_sum(x * attn, batch, 1024) # [1024, 256]

Strategy (v2, bf16):
  - batch is sorted, so shard ROWS at graph boundaries: core c gets all rows
    with batch in [128c, 128(c+1)).  Each core owns exactly 128 output graphs
    -> no cross-core reduction for pooled.
  - b2 cancels in softmax (constant shift) -> dropped.
  - softmax normalizer: each core returns unnormalized A_g = sum_i e_i x_i and
    per-row e values; host divides by the global Z (exact).
  - x is DMA'd HBM->SBUF with an on-the-fly fp32->bf16 cast (SWDGE), so all
    PE work runs in bf16 (FWL weight loads, 1 cyc/row matmuls).
  - h is computed with rows i on PSUM partitions (lhsT = xT tile chunks), so
    the score dot product s_i = w2 . tanh(h_i) is a DVE multiply-reduce along
    the free dim -- no PE score matmuls, and e lands as a [128,1] column.
  - per 128-row tile:
      xT   = transpose(x_tile) on PE (bf16)     (PSUM->SBUF copy DVE+ACT)
      h    = sum_dc xT_dc^T @ W1[dc]            (PE, rhs = W1 moving)
      th   = tanh(h) on ACT -> bf16 SBUF
      s    = reduce_add(th * w2rep) on DVE -> [128,1]
      e    = exp(s) on ACT (batched per 2 tiles) -> evec column
      m    = (iota == brel) * e   one fused DVE tensor_scalar -> bf16
      acc[128g, 256] += m^T @ x_tile   (PE, PSUM-resident accumulator)
"""

import numpy as np
from contextlib import ExitStack

import concourse.bass as bass
import concourse.bacc as bacc
import concourse.mybir as mybir
import concourse.tile as tile
from concourse.bass_utils import run_bass_kernel_spmd
from concourse.masks import make_identity

F32 = mybir.dt.float32
BF16 = mybir.dt.bfloat16
I32 = mybir.dt.int32

NUM_GRAPHS = 1024
NC = 8
GPC = NUM_GRAPHS // NC  # graphs per core = 128
P = 128
D = 256
ST = 8  # tiles per DMA supertile (1 MiB fp32 read per chunk)
G = 4  # tiles per compute group (tanh/exp batching)


def build_program(R: int, T: int, with_b1: bool) -> bass.Bass:
    assert T % ST == 0 and R == T * P
    nsup = T // ST

    nc = bacc.Bacc("TRN2", target_bir_lowering=False, debug=False)
    xs = nc.declare_dram_parameter("xs", [R, D], F32, isOutput=False)
    brel = nc.declare_dram_parameter("brel", [P, T], F32, isOutput=False)
    w1 = nc.declare_dram_parameter("w1", [D, D], F32, isOutput=False)
    w2r = nc.declare_dram_parameter("w2r", [P, D], F32, isOutput=False)
    if with_b1:
        b1d = nc.declare_dram_parameter("b1d", [1, D], F32, isOutput=False)
    pooled = nc.declare_dram_parameter("pooled", [P, D], F32, isOutput=True)
    evec_out = nc.declare_dram_parameter("evec_out", [P, T], F32, isOutput=True)

    with ExitStack() as ctx:
        tc = ctx.enter_context(tile.TileContext(nc))
        const = ctx.enter_context(tc.tile_pool(name="const", bufs=1))
        xpool = ctx.enter_context(tc.tile_pool(name="x", bufs=4))
        xtpp = ctx.enter_context(tc.tile_pool(name="xtp", bufs=3, space="PSUM"))
        xtsp = ctx.enter_context(tc.tile_pool(name="xts", bufs=7))
        hpp = ctx.enter_context(tc.tile_pool(name="hp", bufs=2, space="PSUM"))
        thp = ctx.enter_context(tc.tile_pool(name="th", bufs=4))
        scrp = ctx.enter_context(tc.tile_pool(name="scr", bufs=6))
        spp = ctx.enter_context(tc.tile_pool(name="sp", bufs=6))
        mpl = ctx.enter_context(tc.tile_pool(name="m", bufs=10))
        accp = ctx.enter_context(tc.tile_pool(name="acc", bufs=1, space="PSUM"))
        outp = ctx.enter_context(tc.tile_pool(name="out", bufs=1))

        # ---- constants ----
        identf = const.tile([P, P], F32, tag="identf")
        make_identity(nc, identf[:])
        ident = const.tile([P, P], BF16)
        nc.vector.tensor_copy(ident[:], identf[:])
        iota_i = const.tile([P, P], I32)
        nc.gpsimd.iota(iota_i[:], pattern=[[1, P]], base=0, channel_multiplier=0)
        iota_b = const.tile([P, P], BF16)
        nc.vector.tensor_copy(iota_b[:], iota_i[:])

        w1f = const.tile([P, 2, D], F32, tag="w1f")  # [d_lo, dc, j]
        nc.sync.dma_start(w1f[:], w1.rearrange("(dc p) j -> p dc j", p=P))
        w1sb = const.tile([P, 2, D], BF16)
        nc.vector.tensor_copy(w1sb[:], w1f[:])
        w2f = const.tile([P, D], F32, tag="w2f")  # w2[j] replicated on partitions
        nc.sync.dma_start(w2f[:], w2r[:])
        w2rep = const.tile([P, D], BF16)
        nc.vector.tensor_copy(w2rep[:], w2f[:])
        brelsb = const.tile([P, T], F32)
        nc.sync.dma_start(brelsb[:], brel[:])
        if with_b1:
            b1f = const.tile([1, D], F32, tag="b1f")  # [1, j]
            nc.sync.dma_start(b1f[:], b1d[:])
            b1sb = const.tile([1, D], BF16)
            nc.vector.tensor_copy(b1sb[:], b1f[:])
            ones_cf = const.tile([1, P], F32, tag="ones_cf")
            nc.gpsimd.memset(ones_cf[:], 1.0)
            ones_col = const.tile([1, P], BF16)
            nc.vector.tensor_copy(ones_col[:], ones_cf[:])

        evec = const.tile([P, T], F32, tag="evec")  # exp(s) per row
        # persistent PSUM accumulator
        acc = accp.tile([P, D], F32)  # pooled[g, d]

        Tanh = mybir.ActivationFunctionType.Tanh
        Exp = mybir.ActivationFunctionType.Exp

        def emit_macc(pend):
            """m-build + acc matmuls for a completed group (1-group delay
            keeps the in-order PE queue from head-of-line blocking on m)."""
            gt0_, xsb_, grp_ = pend
            for tt in range(G):
                t = grp_ * G + tt
                gt = gt0_ + tt
                m = mpl.tile([P, P], BF16)
                nc.vector.tensor_scalar(
                    m[:],
                    iota_b[:],
                    brelsb[:, gt : gt + 1],
                    evec[:, gt : gt + 1],
                    op0=mybir.AluOpType.is_equal,
                    op1=mybir.AluOpType.mult,
                )
                nc.tensor.matmul(
                    acc[:],
                    lhsT=m[:],
                    rhs=xsb_[:, t, :],
                    start=(gt == 0),
                    stop=(gt == T - 1),
                    skip_group_check=True,
                )

        for sup in range(nsup):
            xsb = xpool.tile([P, ST, D], BF16)
            src = xs[sup * ST * P : (sup + 1) * ST * P, :]
            nc.gpsimd.dma_start(xsb[:], src.rearrange("(p t) d -> p t d", p=P))

            for grp in range(ST // G):
                s4 = spp.tile([P, G], F32)
                h4 = hpp.tile([P, G, D], F32)
                th4 = thp.tile([P, G, D], BF16)
                for tt in range(G):
                    t = grp * G + tt
                    # transpose x_tile -> xT chunks in PSUM (bf16)
                    xtp = xtpp.tile([P, 2, P], BF16)
                    for dc in range(2):
                        nc.tensor.transpose(
                            xtp[:, dc, :],
                            xsb[:, t, dc * P : (dc + 1) * P],
                            ident[:],
                        )
                    xts = xtsp.tile([P, 2, P], BF16)
                    nc.vector.tensor_copy(xts[:], xtp[:])

                    # h[i, j] = sum_d x[i, d] W1[d, j]  (i on partitions)
                    for dc in range(2):
                        nc.tensor.matmul(
                            h4[:, tt, :],
                            lhsT=xts[:, dc],
                            rhs=w1sb[:, dc],
                            start=(dc == 0),
                            stop=(dc == 1 and not with_b1),
                        )
                    if with_b1:
                        nc.tensor.matmul(
                            h4[:, tt, :],
                            lhsT=ones_col[:],
                            rhs=b1sb[:],
                            start=False,
                            stop=True,
                        )
                # tanh per PSUM bank (2 tiles = 2KB; a 2-bank AP reads wrong)
                nc.scalar.activation(th4[:, 0:2], h4[:, 0:2], Tanh)
                nc.scalar.activation(th4[:, 2:4], h4[:, 2:4], Tanh)

                for tt in range(G):
                    # s_i = sum_j th[i, j] * w2[j]: fused mult+reduce on DVE
                    scr = scrp.tile([P, D], BF16)
                    nc.vector.scalar_tensor_tensor(
                        scr[:],
                        th4[:, tt, :],
                        1.0,
                        w2rep[:],
                        op0=mybir.AluOpType.mult,
                        op1=mybir.AluOpType.mult,
                        accum_out=s4[:, tt : tt + 1],
                    )

                gt0 = sup * ST + grp * G
                nc.scalar.activation(evec[:, gt0 : gt0 + G], s4[:], Exp)

                emit_macc((gt0, xsb, grp))

        out_sb = outp.tile([P, D], F32)
        nc.vector.tensor_copy(out_sb[:], acc[:])
        nc.sync.dma_start(pooled[:], out_sb[:])
        nc.sync.dma_start(evec_out[:], evec[:])

    nc.compile()
    return nc


def _prep_inputs(x, batch, W1, b1, W2):
    """Shard rows at graph boundaries; pad to a common multiple of ST*P rows."""
    x = np.ascontiguousarray(np.asarray(x, dtype=np.float32))
    batch = np.asarray(batch)
    bounds = np.searchsorted(batch, np.arange(0, NUM_GRAPHS + 1, GPC))
    counts = np.diff(bounds)
    chunk = ST * P
    R = int(np.ceil(max(int(counts.max()), 1) / chunk) * chunk)
    T = R // P

    w1h = np.ascontiguousarray(np.asarray(W1, dtype=np.float32))  # [d, j]
    w2h = np.ascontiguousarray(
        np.broadcast_to(np.asarray(W2, dtype=np.float32).reshape(1, D), (P, D))
    )  # w2 replicated across partitions
    b1h = np.asarray(b1, dtype=np.float32).reshape(1, D)
    with_b1 = bool(np.any(b1h))

    in_maps = []
    for c in range(NC):
        lo, hi = int(bounds[c]), int(bounds[c + 1])
        n = hi - lo
        xs = np.zeros((R, D), dtype=np.float32)
        xs[:n] = x[lo:hi]
        br = np.full((R,), -1.0, dtype=np.float32)
        br[:n] = (np.asarray(batch[lo:hi], dtype=np.int64) - c * GPC).astype(
            np.float32
        )
        # row r = sup*(P*ST) + p*ST + t lives at brel_pt[p, sup*ST + t]
        nsup = T // ST
        brel_pt = np.ascontiguousarray(
            br.reshape(nsup, P, ST).transpose(1, 0, 2).reshape(P, T)
        )
        m = {"xs": xs, "brel": brel_pt, "w1": w1h, "w2r": w2h}
        if with_b1:
            m["b1d"] = b1h
        in_maps.append(m)
    return in_maps, R, T, with_b1, [int(c) for c in counts]


def run(x, batch, W1, b1, W2, b2, trace=False, trace_kwargs=None):
    in_maps, R, T, with_b1, counts = _prep_inputs(x, batch, W1, b1, W2)
    nc = build_program(R, T, with_b1)
    res = run_bass_kernel_spmd(
        nc,
        in_maps,
        core_ids=list(range(NC)),
        trace=trace,
        **(trace_kwargs or {}),
    )
    A = np.concatenate(
        [res.results[c]["pooled"] for c in range(NC)], axis=0
    ).astype(np.float64)
    nsup = T // ST
    Z = 0.0
    for c in range(NC):
        ev = res.results[c]["evec_out"].astype(np.float64)  # [P, T]
        n = counts[c]
        # row r = sup*(P*ST) + p*ST + t is at ev[p, sup*ST + t]
        rows = ev.reshape(P, nsup, ST).transpose(1, 0, 2).reshape(-1)
        Z += rows[:n].sum()
    out = (A / Z).astype(np.float32)
    return out, res


def kernel(x, batch, W1, b1, W2, b2):
    out, _ = run(x, batch, W1, b1, W2, b2)
    return out
